# revision 4
# baseline (speedup 1.0000x reference)
"""Trainium2 Bass kernel for nn_MoELayer (moe_routing).

Strategy: data-parallel over tokens (1024 tokens/core on 8 cores), expert
weights replicated per core (host-cast to bf16 to halve HBM traffic).
Per core:
  phase 1: PE-transpose x -> xT (fp32 for router, bf16 for expert matmuls),
           fp32 router matmul, softmax (ACT exp), top-2 via DVE max/max_index,
           combine-weight matrix cw, aux-loss partials via ones^T matmuls.
  phase 2: per expert e: h = gelu(w1[e]^T x) feature-major via
           matmul(lhsT=w1 chunks, rhs=xT), then token-major
           y = matmul(lhsT=h chunks, rhs=w2 chunks) and a fused
           out_acc = psum*cw[:,e] + out_acc (scalar_tensor_tensor FMA).
aux_loss is finished on host from per-core partial sums (part of unsharding).
"""

import numpy as np

import concourse.bacc as bacc
import concourse.mybir as mybir
import concourse.tile as tile
from concourse import bass_utils
from concourse.masks import make_identity

P = 128
N_CORES = 8
B, T, D = 4, 2048, 1024
E, H, K = 8, 2048, 2
N_TOK = B * T                 # 8192
T_SH = N_TOK // N_CORES       # 1024 tokens per core
N_TT = T_SH // P              # 8 token tiles per core
N_DC = D // P                 # 8 D chunks
N_HC = H // P                 # 16 H chunks
NBLK = 512
NB_TOK = T_SH // NBLK         # 2 token blocks (mm1 moving dim)
NB_D = D // NBLK              # 2 D blocks (mm2 moving dim)

FP = mybir.dt.float32
BF = mybir.dt.bfloat16
U32 = mybir.dt.uint32

AF = mybir.ActivationFunctionType
ALU = mybir.AluOpType
AX = mybir.AxisListType


def build():
    nc = bacc.Bacc("TRN2", target_bir_lowering=False, debug=False,
                   num_devices=N_CORES)

    xs = nc.dram_tensor("xs", [T_SH, D], FP, kind="ExternalInput")
    gw = nc.dram_tensor("gw", [D, E], FP, kind="ExternalInput")
    w1 = nc.dram_tensor("w1", [E, D, H], BF, kind="ExternalInput")
    w2 = nc.dram_tensor("w2", [E, H, D], BF, kind="ExternalInput")

    out_s = nc.dram_tensor("out_s", [T_SH, D], FP, kind="ExternalOutput")
    probs_s = nc.dram_tensor("probs_s", [T_SH, E], FP, kind="ExternalOutput")
    tki_s = nc.dram_tensor("tki_s", [T_SH, K], U32, kind="ExternalOutput")
    tkp_s = nc.dram_tensor("tkp_s", [T_SH, K], FP, kind="ExternalOutput")
    stats_s = nc.dram_tensor("stats_s", [1, 2 * E], FP, kind="ExternalOutput")

    with tile.TileContext(nc) as tc:
        with (
            tc.tile_pool(name="const", bufs=1) as constp,
            tc.tile_pool(name="persist", bufs=1) as pers,
        ):
            ident = constp.tile([P, P], FP)
            make_identity(nc, ident[:])
            ones = constp.tile([P, 1], FP)
            nc.vector.memset(ones[:], 1.0)
            gw_sb = constp.tile([P, N_DC, E], FP)
            nc.sync.dma_start(gw_sb[:], gw.ap().rearrange("(c p) e -> p c e", p=P))

            xT_bf = pers.tile([P, N_DC, T_SH], BF)      # 2 MiB
            cw = pers.tile([P, N_TT, E], FP)            # combine weights
            out_acc = pers.tile([P, N_TT, D], FP)       # 4 MiB

            # ---------------- phase 1: transpose + router ----------------
            with (
                tc.tile_pool(name="p1", bufs=3) as p1,
                tc.tile_pool(name="p1ps", bufs=4, space="PSUM") as p1ps,
                tc.tile_pool(name="p1psl", bufs=2, space="PSUM") as p1psl,
                tc.tile_pool(name="statps", bufs=1, space="PSUM") as statps,
            ):
                # separate tiles (separate PSUM banks): start=True clears
                # has_written at bank granularity, so accumulation groups
                # must not share a bank
                ps_cnt = statps.tile([1, E], FP, tag="cnt")
                ps_psm = statps.tile([1, E], FP, tag="psm")
                for tt in range(N_TT):
                    tsl = slice(tt * P, (tt + 1) * P)
                    x_nat = p1.tile([P, D], FP, tag="xnat")
                    nc.sync.dma_start(x_nat[:], xs.ap()[tsl, :])
                    xT_f = p1.tile([P, N_DC, P], FP, tag="xtf")
                    for dc in range(N_DC):
                        ps_t = p1ps.tile([P, P], FP, tag="pst")
                        nc.tensor.transpose(ps_t[:], x_nat[:, dc * P:(dc + 1) * P],
                                            ident[:])
                        nc.vector.tensor_copy(xT_f[:, dc, :], ps_t[:])
                        nc.vector.tensor_copy(xT_bf[:, dc, tsl], ps_t[:])
                    ps_l = p1psl.tile([P, E], FP, tag="psl")
                    for dc in range(N_DC):
                        nc.tensor.matmul(ps_l[:], xT_f[:, dc, :], gw_sb[:, dc, :],
                                         start=(dc == 0), stop=(dc == N_DC - 1))
                    lg = p1.tile([P, E], FP, tag="lg")
                    nc.vector.tensor_copy(lg[:], ps_l[:])

                    # softmax probs = exp(lg - max) / sum
                    neg_m = p1.tile([P, 1], FP, tag="negm")
                    nc.vector.tensor_reduce(neg_m[:], lg[:], axis=AX.X,
                                            op=ALU.max, negate=True)
                    probs_t = p1.tile([P, E], FP, tag="probs")
                    ssum = p1.tile([P, 1], FP, tag="ssum")
                    nc.scalar.activation(probs_t[:], lg[:], AF.Exp,
                                         bias=neg_m[:], scale=1.0,
                                         accum_out=ssum[:])
                    rs = p1.tile([P, 1], FP, tag="rs")
                    nc.vector.reciprocal(rs[:], ssum[:])
                    nc.vector.tensor_scalar_mul(probs_t[:], probs_t[:], rs[:])
                    nc.sync.dma_start(probs_s.ap()[tsl, :], probs_t[:])

                    # top-2 selection on fp32 logits (same order as probs)
                    v8 = p1.tile([P, 8], FP, tag="v8")
                    nc.vector.max(v8[:], lg[:])
                    i8 = p1.tile([P, 8], U32, tag="i8")
                    nc.vector.max_index(i8[:], v8[:], lg[:])
                    nc.sync.dma_start(tki_s.ap()[tsl, :], i8[:, 0:K])

                    # renormalized top-2 probs
                    pe2 = p1.tile([P, K], FP, tag="pe2")
                    nc.scalar.activation(pe2[:], v8[:, 0:K], AF.Exp,
                                         bias=neg_m[:], scale=1.0)
                    nc.vector.tensor_scalar_mul(pe2[:], pe2[:], rs[:])
                    den = p1.tile([P, 1], FP, tag="den")
                    nc.vector.tensor_add(den[:], pe2[:, 0:1], pe2[:, 1:2])
                    rden = p1.tile([P, 1], FP, tag="rden")
                    nc.vector.reciprocal(rden[:], den[:])
                    tkp_t = p1.tile([P, K], FP, tag="tkp")
                    nc.vector.tensor_scalar_mul(tkp_t[:], pe2[:], rden[:])
                    nc.sync.dma_start(tkp_s.ap()[tsl, :], tkp_t[:])

                    # cw[:, tt, e] = probs * rden where lg >= second max, else 0
                    mask = p1.tile([P, E], FP, tag="mask")
                    nc.vector.tensor_scalar(mask[:], lg[:], v8[:, 1:2], None,
                                            op0=ALU.is_ge)
                    nc.vector.scalar_tensor_tensor(
                        out=cw[:, tt, :], in0=probs_t[:], scalar=rden[:],
                        in1=mask[:], op0=ALU.mult, op1=ALU.mult)

                    # aux-loss partials: counts and prob sums over tokens
                    nc.tensor.matmul(ps_cnt[:], ones[:], mask[:],
                                     start=(tt == 0), stop=(tt == N_TT - 1))
                    nc.tensor.matmul(ps_psm[:], ones[:], probs_t[:],
                                     start=(tt == 0), stop=(tt == N_TT - 1))
                stat_sb = p1.tile([1, 2 * E], FP, tag="statsb")
                nc.vector.tensor_copy(stat_sb[:, 0:E], ps_cnt[:])
                nc.vector.tensor_copy(stat_sb[:, E:2 * E], ps_psm[:])
                nc.sync.dma_start(stats_s.ap()[:, :], stat_sb[:])

            # ---------------- phase 2: experts ----------------
            with (
                tc.tile_pool(name="wp1", bufs=1) as wp1,
                tc.tile_pool(name="wp2", bufs=1) as wp2,
                tc.tile_pool(name="hp", bufs=1) as hp,
                tc.tile_pool(name="ps1", bufs=2, space="PSUM") as ps1p,
                tc.tile_pool(name="ps2", bufs=2, space="PSUM") as ps2p,
            ):
                for e in range(E):
                    w1_sb = wp1.tile([P, N_DC, H], BF, tag="w1")
                    nc.sync.dma_start(
                        w1_sb[:],
                        w1.ap()[e].rearrange("(c p) h -> p c h", p=P))
                    w2_sb = wp2.tile([P, N_HC, D], BF, tag="w2")
                    nc.sync.dma_start(
                        w2_sb[:],
                        w2.ap()[e].rearrange("(c p) d -> p c d", p=P))

                    h_sb = hp.tile([P, N_HC, T_SH], BF, tag="h")
                    for mt in range(N_HC):
                        for nb in range(NB_TOK):
                            nsl = slice(nb * NBLK, (nb + 1) * NBLK)
                            ps = ps1p.tile([P, NBLK], FP, tag="ps1")
                            for dc in range(N_DC):
                                nc.tensor.matmul(
                                    ps[:],
                                    w1_sb[:, dc, mt * P:(mt + 1) * P],
                                    xT_bf[:, dc, nsl],
                                    start=(dc == 0), stop=(dc == N_DC - 1))
                            nc.scalar.activation(h_sb[:, mt, nsl], ps[:], AF.Gelu)

                    for tt in range(N_TT):
                        for db in range(NB_D):
                            dsl = slice(db * NBLK, (db + 1) * NBLK)
                            ps2 = ps2p.tile([P, NBLK], FP, tag="ps2")
                            for hc in range(N_HC):
                                nc.tensor.matmul(
                                    ps2[:],
                                    h_sb[:, hc, tt * P:(tt + 1) * P],
                                    w2_sb[:, hc, dsl],
                                    start=(hc == 0), stop=(hc == N_HC - 1))
                            if e == 0:
                                nc.vector.tensor_scalar_mul(
                                    out_acc[:, tt, dsl], ps2[:],
                                    cw[:, tt, e:e + 1])
                            else:
                                nc.vector.scalar_tensor_tensor(
                                    out=out_acc[:, tt, dsl], in0=ps2[:],
                                    scalar=cw[:, tt, e:e + 1],
                                    in1=out_acc[:, tt, dsl],
                                    op0=ALU.mult, op1=ALU.add)

            nc.sync.dma_start(out_s.ap().rearrange("(t p) d -> p t d", p=P),
                              out_acc[:])

    nc.compile()
    return nc


_nc = None


def _get_nc():
    global _nc
    if _nc is None:
        _nc = build()
    return _nc


def kernel(x, gate_w, w1, w2):
    nc = _get_nc()
    bf_np = mybir.dt.np(BF)

    xf = np.ascontiguousarray(np.asarray(x, np.float32).reshape(N_TOK, D))
    gwf = np.ascontiguousarray(np.asarray(gate_w, np.float32))
    w1b = np.ascontiguousarray(np.asarray(w1, np.float32).astype(bf_np))
    w2b = np.ascontiguousarray(np.asarray(w2, np.float32).astype(bf_np))

    in_maps = []
    for c in range(N_CORES):
        in_maps.append({
            "xs": np.ascontiguousarray(xf[c * T_SH:(c + 1) * T_SH]),
            "gw": gwf,
            "w1": w1b,
            "w2": w2b,
        })

    res = bass_utils.run_bass_kernel_spmd(
        nc, in_maps, core_ids=list(range(N_CORES)))
    outs = res.results

    out = np.concatenate([outs[c]["out_s"] for c in range(N_CORES)], 0)
    out = out.reshape(B, T, D)
    probs = np.concatenate([outs[c]["probs_s"] for c in range(N_CORES)], 0)
    probs = probs.reshape(B, T, E)
    tki = np.concatenate([outs[c]["tki_s"] for c in range(N_CORES)], 0)
    tki = tki.astype(np.int32).reshape(B, T, K)
    tkp = np.concatenate([outs[c]["tkp_s"] for c in range(N_CORES)], 0)
    tkp = tkp.reshape(B, T, K)

    stats = np.stack([outs[c]["stats_s"][0] for c in range(N_CORES)], 0)
    stats = stats.astype(np.float64).sum(0)
    counts, psums = stats[:E], stats[E:]
    aux = np.float32(E * np.sum((counts / N_TOK) * (psums / N_TOK)))

    return (out, aux, probs, tki, tkp)
